# revision 23
# baseline (speedup 1.0000x reference)
"""Self pairwise Euclidean distance on Trainium2 (8 NeuronCores),
exploiting output symmetry at 128-row granularity.

out[i, j] = ||x[j] - x[i]||_2 for x of shape [8192, 64] fp32. The output
is symmetric, so each unordered pair {i, j} only needs to be computed
once on-device; the host mirrors block transposes while unsharding.

Row sharding is block-cyclic at 128-row granularity: core c owns the 8
row blocks c, c+8, ..., c+56 (of 64). For the block starting at row s,
the device computes distance columns (s + [0, 4224)) mod N — its own
diagonal columns plus the next N/2. Every pair {i, j} with
(j - i) mod N <= 4096 appears in row i's window, or with >= 4096 in row
j's window, so the union of windows covers every pair; the host fills
the remaining 1984 of 4096 [128, 128] blocks with transposes of
computed blocks. Per core the 8 windows are 1024-col shifts of each
other, so one wrap-padded [66, 11392] B window serves all 8 row tiles.

Per-core device program (identical on every core; per-core inputs
differ): the contraction dim is augmented twice so one matmul chain per
tile produces the COMPLETE d2 in PSUM:
  A = [x_rows^T; ones; sqn_i]  (K=66, M=128)
  B = [-2*x^T;   sqn_j; ones]  (K=66, padded window)
  => psum = -2*gram + sqn_j + sqn_i = d2.
Each PSUM group ([128, 1536] or the [128, 1152] tail) goes through a
bias-free ScalarE Sqrt (the only engine with sqrt) into an fp16 tile
DMA'd out per group. Group 0 starts on the diagonal (the only entries
whose d2 can go fp-negative; min off-diagonal d2 is ~30.6) and is
processed LAST in each row: VectorE clamps the leading 128-wide
diagonal sub-block in place with max(psum, 0). The host overwrites the
exact diagonal with 0.

Startup hiding: a dummy activation pulls the ~1.3us sqrt-table load to
t~0, and a chain of throwaway matmuls keeps the Tensor engine busy
through the input load so the first real matmuls run at full clock.
"""

import numpy as np

N = 8192
D = 64
K = D + 2  # contraction: 64 data rows + ones (sqn_j) + sqn_i (ones)
NCORES = 8
PT = 128  # row-block granularity == partition tile
NBLK = N // PT  # 64
RPC = 1024  # rows per core (8 blocks, stride 8*PT)
W = N // 2 + PT  # 4224-wide computed window per row block
NT_M = RPC // PT  # 8 row tiles per core
BW = (NT_M - 1) * NCORES * PT + W  # 11392: padded B span per core
CT = 512  # matmul free-dim chunk (one PSUM bank)
GT = 1536  # psum group (3 banks)
GSIZES = [1152, 1536, 1536]  # groups g2(tail), g1, g0(diag) per row
GOFFS = [3072, 1536, 0]  # window-local offsets, processed in this order

_NC_CACHE = {}


def _build_nc():
    import concourse.mybir as mybir
    import concourse.tile as tile
    from concourse import bacc

    f32 = mybir.dt.float32
    f32r = mybir.dt.float32r
    f16 = mybir.dt.float16
    bf16 = mybir.dt.bfloat16
    AF = mybir.ActivationFunctionType

    nc = bacc.Bacc(
        "TRN2",
        target_bir_lowering=False,
        debug=False,
        num_devices=NCORES,
    )
    bt = nc.dram_tensor("bt", [K, BW], f32r, kind="ExternalInput").ap()
    at = nc.dram_tensor("at", [K, RPC], f32r, kind="ExternalInput").ap()
    out = nc.dram_tensor("out", [RPC, W], f16, kind="ExternalOutput").ap()

    with tile.TileContext(nc) as tc:
        with (
            tc.tile_pool(name="persist", bufs=1) as persist,
            tc.tile_pool(name="outp", bufs=6) as outp,
            tc.tile_pool(name="ps", bufs=2, space="PSUM") as psp,
            tc.tile_pool(name="psw", bufs=1, space="PSUM") as psw,
        ):
            B = persist.tile([K, BW], f32r)
            A = persist.tile([K, RPC], f32r)
            SCR = persist.tile([2, CT // 2], bf16)
            SCF = persist.tile([1, 2], f32)

            # Dummy activation up front: pulls the ~1.3us sqrt table load
            # (inserted before the first activation) off the critical path.
            nc.vector.memset(SCF[:, :], 1.0)
            nc.scalar.activation(SCF[:, 1:2], SCF[:, 0:1], AF.Sqrt)
            nc.vector.memset(SCR[:, :], 1.0)

            # Input loads, ordered by first consumption: row 0 consumes
            # window cols [1536:3072), [3072:4224), [0:1536); later rows
            # extend the tail (windows overlap by 3200 cols per step).
            nc.sync.dma_start(A[:, :PT], at[:, :PT])
            chunks = [(3072, 4224), (1536, 3072), (0, 1536), (4224, 6272)]
            for lo in range(6272, BW, 2048):
                chunks.append((lo, min(lo + 2048, BW)))
            for i, (lo, hi) in enumerate(chunks):
                nc.sync.dma_start(B[:, lo:hi], bt[:, lo:hi])
                if i == 1:
                    nc.sync.dma_start(A[:, PT:], at[:, PT:])

            # PE warmup chain (see module docstring).
            wps = psw.tile([1, CT // 2], f32)
            for _ in range(12):
                nc.tensor.matmul(
                    wps[:, :], SCR[:, 0:1], SCR[:, :], start=True, stop=True
                )

            for m in range(NT_M):
                w0 = m * NCORES * PT  # window start within the padded B
                for gi, (goff, gsz) in enumerate(zip(GOFFS, GSIZES)):
                    last = m == NT_M - 1 and gi == len(GOFFS) - 1
                    ps = psp.tile([PT, GT], f32)
                    for j in range(0, gsz, CT):
                        cw = min(CT, gsz - j)
                        nc.tensor.matmul(
                            ps[:, j : j + cw],
                            A[:, m * PT : (m + 1) * PT],
                            B[:, w0 + goff + j : w0 + goff + j + cw],
                            start=True,
                            stop=True,
                        )
                    if goff == 0:
                        # Clamp the 128-wide diagonal sub-block in place.
                        nc.vector.tensor_scalar_max(
                            ps[:, :PT], ps[:, :PT], 0.0
                        )
                    ot = outp.tile([PT, GT], f16)
                    orow = out[m * PT : (m + 1) * PT, :]
                    if not last:
                        nc.scalar.activation(ot[:, :gsz], ps[:, :gsz], AF.Sqrt)
                        nc.sync.dma_start(
                            orow[:, goff : goff + gsz], ot[:, :gsz]
                        )
                    else:
                        # Final group: split 1024+512 so the very last DMA is
                        # half-size (shorter critical-path tail).
                        h = 1024
                        nc.scalar.activation(ot[:, :h], ps[:, :h], AF.Sqrt)
                        nc.sync.dma_start(orow[:, goff : goff + h], ot[:, :h])
                        nc.scalar.activation(ot[:, h:gsz], ps[:, h:gsz], AF.Sqrt)
                        nc.sync.dma_start(
                            orow[:, goff + h : goff + gsz], ot[:, h:gsz]
                        )
    nc.compile()
    return nc


def _get_nc():
    if "nc" not in _NC_CACHE:
        _NC_CACHE["nc"] = _build_nc()
    return _NC_CACHE["nc"]


def _round_fp32r(a: np.ndarray) -> np.ndarray:
    """Round fp32 to the fp32r grid (E8M11, round-to-nearest-even)."""
    u = np.ascontiguousarray(a, dtype=np.float32).view(np.uint32)
    r = (u + np.uint32(0x7FF) + ((u >> np.uint32(12)) & np.uint32(1))) & np.uint32(
        0xFFFFF000
    )
    return r.view(np.float32)


def _core_rows(c: int) -> np.ndarray:
    """Global row indices owned by core c (blocks c, c+8, ..., c+56)."""
    return (
        np.arange(NT_M)[:, None] * (NCORES * PT)
        + c * PT
        + np.arange(PT)[None, :]
    ).reshape(-1)


def _prep_inputs(x: np.ndarray):
    xt = np.ascontiguousarray(x.T)
    sqn = np.einsum("nd,nd->n", x, x).astype(np.float32)
    ones = np.ones((1, N), np.float32)
    bt = _round_fp32r(np.vstack([-2.0 * xt, sqn[None, :], ones]))
    amat = _round_fp32r(np.vstack([xt, ones, sqn[None, :]]))
    idx = np.arange(BW)
    in_maps = []
    for c in range(NCORES):
        rows = _core_rows(c)
        in_maps.append(
            {
                "bt": np.ascontiguousarray(bt[:, (c * PT + idx) % N]),
                "at": np.ascontiguousarray(amat[:, rows]),
            }
        )
    return in_maps


def _assemble(blocks) -> np.ndarray:
    """Place each core's [1024, 4224] fp16 bands, then mirror the rest."""
    full = np.empty((N, N), dtype=np.float32)
    for c, blk in enumerate(blocks):
        for m in range(NT_M):
            start = m * NCORES * PT + c * PT
            rows = slice(start, start + PT)
            part = blk[m * PT : (m + 1) * PT, :]
            end = start + W
            if end <= N:
                full[rows, start:end] = part
            else:
                k = N - start
                full[rows, start:] = part[:, :k]
                full[rows, : end - N] = part[:, k:]
    # Mirror the uncomputed [128, 128] blocks from their transposes.
    nw = W // PT  # 33 directly-written blocks per block row
    for r in range(NBLK):
        for s in range(NBLK):
            if (s - r) % NBLK >= nw:
                full[r * PT : (r + 1) * PT, s * PT : (s + 1) * PT] = full[
                    s * PT : (s + 1) * PT, r * PT : (r + 1) * PT
                ].T
    np.fill_diagonal(full, 0.0)
    return full


def _run(inputs, trace=False, trace_cores=None):
    from concourse.bass_utils import run_bass_kernel_spmd

    x = np.ascontiguousarray(np.asarray(inputs["x"], dtype=np.float32))
    assert x.shape == (N, D), x.shape
    in_maps = _prep_inputs(x)
    res = run_bass_kernel_spmd(
        _get_nc(),
        in_maps,
        core_ids=list(range(NCORES)),
        trace=trace,
        trace_cores=trace_cores,
    )
    full = _assemble([r["out"] for r in res.results])
    return full, res


def kernel(**inputs) -> np.ndarray:
    full, _ = _run(inputs)
    return full


# revision 26
# speedup vs baseline: 1.0211x; 1.0211x over previous
"""Self pairwise Euclidean distance on Trainium2 (8 NeuronCores),
exploiting output symmetry at 128-row granularity.

out[i, j] = ||x[j] - x[i]||_2 for x of shape [8192, 64] fp32. The output
is symmetric, so each unordered pair {i, j} only needs to be computed
once on-device; the host mirrors block transposes while unsharding.

Row sharding is block-cyclic at 128-row granularity: core c owns the 8
row blocks c, c+8, ..., c+56 (of 64). For the block starting at row s,
the device computes distance columns (s + [0, 4224)) mod N — its own
diagonal columns plus the next N/2. Every pair {i, j} with
(j - i) mod N <= 4096 appears in row i's window, or with >= 4096 in row
j's window, so the union of windows covers every pair; the host fills
the remaining 1984 of 4096 [128, 128] blocks with transposes of
computed blocks. Per core the 8 windows are 1024-col shifts of each
other, so one wrap-padded [66, 11392] B window serves all 8 row tiles.

Per-core device program (identical on every core; per-core inputs
differ): the contraction dim is augmented twice so one matmul chain per
tile produces the COMPLETE d2 in PSUM:
  A = [x_rows^T; ones; sqn_i]  (K=66, M=128)
  B = [-2*x^T;   sqn_j; ones]  (K=66, padded window)
  => psum = -2*gram + sqn_j + sqn_i = d2.
Each PSUM group ([128, 1536] or the [128, 1152] tail) goes through a
bias-free ScalarE Sqrt (the only engine with sqrt) into an fp16 tile
DMA'd out per group. Group 0 starts on the diagonal (the only entries
whose d2 can go fp-negative; min off-diagonal d2 is ~30.6) and is
processed LAST in each row: VectorE clamps the leading 128-wide
diagonal sub-block in place with max(psum, 0). The host overwrites the
exact diagonal with 0.

Startup hiding: a dummy activation pulls the ~1.3us sqrt-table load to
t~0, and a chain of throwaway matmuls keeps the Tensor engine busy
through the input load so the first real matmuls run at full clock.
"""

import numpy as np

N = 8192
D = 64
K = D + 2  # contraction: 64 data rows + ones (sqn_j) + sqn_i (ones)
NCORES = 8
PT = 128  # row-block granularity == partition tile
NBLK = N // PT  # 64
RPC = 1024  # rows per core (8 blocks, stride 8*PT)
W = N // 2 + PT  # 4224-wide computed window per row block
NT_M = RPC // PT  # 8 row tiles per core
BW = (NT_M - 1) * NCORES * PT + W  # 11392: padded B span per core
CT = 512  # matmul free-dim chunk (one PSUM bank)
GT = 1536  # psum group (3 banks)
GSIZES = [1152, 1536, 1536]  # groups g2(tail), g1, g0(diag) per row
GOFFS = [3072, 1536, 0]  # window-local offsets, processed in this order

_NC_CACHE = {}


def _build_nc():
    import concourse.mybir as mybir
    import concourse.tile as tile
    from concourse import bacc

    f32 = mybir.dt.float32
    f32r = mybir.dt.float32r
    f16 = mybir.dt.float16
    bf16 = mybir.dt.bfloat16
    AF = mybir.ActivationFunctionType

    nc = bacc.Bacc(
        "TRN2",
        target_bir_lowering=False,
        debug=False,
        num_devices=NCORES,
    )
    bt = nc.dram_tensor("bt", [K, BW], bf16, kind="ExternalInput").ap()
    at = nc.dram_tensor("at", [K, RPC], bf16, kind="ExternalInput").ap()
    out = nc.dram_tensor("out", [RPC, W], f16, kind="ExternalOutput").ap()

    with tile.TileContext(nc) as tc:
        with (
            tc.tile_pool(name="persist", bufs=1) as persist,
            tc.tile_pool(name="outp", bufs=6) as outp,
            tc.tile_pool(name="ps", bufs=2, space="PSUM") as psp,
            tc.tile_pool(name="psw", bufs=1, space="PSUM") as psw,
        ):
            B = persist.tile([K, BW], bf16)
            A = persist.tile([K, RPC], bf16)
            SCR = persist.tile([2, CT // 2], bf16)
            SCF = persist.tile([1, 2], f32)

            # Dummy activation up front: pulls the ~1.3us sqrt table load
            # (inserted before the first activation) off the critical path.
            nc.vector.memset(SCF[:, :], 1.0)
            nc.scalar.activation(SCF[:, 1:2], SCF[:, 0:1], AF.Sqrt)
            nc.vector.memset(SCR[:, :], 1.0)

            # Input loads, ordered by first consumption: row 0 consumes
            # window cols [1536:3072), [3072:4224), [0:1536); later rows
            # extend the tail (windows overlap by 3200 cols per step).
            nc.sync.dma_start(A[:, :PT], at[:, :PT])
            chunks = [(3072, 4224), (1536, 3072), (0, 1536), (4224, 6272)]
            for lo in range(6272, BW, 2048):
                chunks.append((lo, min(lo + 2048, BW)))
            for i, (lo, hi) in enumerate(chunks):
                nc.sync.dma_start(B[:, lo:hi], bt[:, lo:hi])
                if i == 1:
                    nc.sync.dma_start(A[:, PT:], at[:, PT:])

            # PE warmup chain (see module docstring).
            wps = psw.tile([1, CT // 2], f32)
            for _ in range(12):
                nc.tensor.matmul(
                    wps[:, :], SCR[:, 0:1], SCR[:, :], start=True, stop=True
                )

            for m in range(NT_M):
                w0 = m * NCORES * PT  # window start within the padded B
                for gi, (goff, gsz) in enumerate(zip(GOFFS, GSIZES)):
                    last = m == NT_M - 1 and gi == len(GOFFS) - 1
                    ps = psp.tile([PT, GT], f32)
                    for j in range(0, gsz, CT):
                        cw = min(CT, gsz - j)
                        nc.tensor.matmul(
                            ps[:, j : j + cw],
                            A[:, m * PT : (m + 1) * PT],
                            B[:, w0 + goff + j : w0 + goff + j + cw],
                            start=True,
                            stop=True,
                        )
                    if goff == 0:
                        # Clamp the 128-wide diagonal sub-block in place.
                        nc.vector.tensor_scalar_max(
                            ps[:, :PT], ps[:, :PT], 0.0
                        )
                    ot = outp.tile([PT, GT], f16)
                    orow = out[m * PT : (m + 1) * PT, :]
                    if not last:
                        nc.scalar.activation(ot[:, :gsz], ps[:, :gsz], AF.Sqrt)
                        nc.sync.dma_start(
                            orow[:, goff : goff + gsz], ot[:, :gsz]
                        )
                    else:
                        # Final group: split 1024+512 so the very last DMA is
                        # half-size (shorter critical-path tail).
                        h = 1024
                        nc.scalar.activation(ot[:, :h], ps[:, :h], AF.Sqrt)
                        nc.sync.dma_start(orow[:, goff : goff + h], ot[:, :h])
                        nc.scalar.activation(ot[:, h:gsz], ps[:, h:gsz], AF.Sqrt)
                        nc.sync.dma_start(
                            orow[:, goff + h : goff + gsz], ot[:, h:gsz]
                        )
    nc.compile()
    return nc


def _get_nc():
    if "nc" not in _NC_CACHE:
        _NC_CACHE["nc"] = _build_nc()
    return _NC_CACHE["nc"]


def _round_fp32r(a: np.ndarray) -> np.ndarray:
    """Round fp32 to the fp32r grid (E8M11, round-to-nearest-even)."""
    u = np.ascontiguousarray(a, dtype=np.float32).view(np.uint32)
    r = (u + np.uint32(0x7FF) + ((u >> np.uint32(12)) & np.uint32(1))) & np.uint32(
        0xFFFFF000
    )
    return r.view(np.float32)


def _core_rows(c: int) -> np.ndarray:
    """Global row indices owned by core c (blocks c, c+8, ..., c+56)."""
    return (
        np.arange(NT_M)[:, None] * (NCORES * PT)
        + c * PT
        + np.arange(PT)[None, :]
    ).reshape(-1)


def _prep_inputs(x: np.ndarray):
    import ml_dtypes

    bf16 = ml_dtypes.bfloat16
    xt = np.ascontiguousarray(x.T)
    sqn = np.einsum("nd,nd->n", x, x).astype(np.float32)
    ones = np.ones((1, N), np.float32)
    bt = np.vstack([-2.0 * xt, sqn[None, :], ones]).astype(bf16)
    amat = np.vstack([xt, ones, sqn[None, :]]).astype(bf16)
    idx = np.arange(BW)
    in_maps = []
    for c in range(NCORES):
        rows = _core_rows(c)
        in_maps.append(
            {
                "bt": np.ascontiguousarray(bt[:, (c * PT + idx) % N]),
                "at": np.ascontiguousarray(amat[:, rows]),
            }
        )
    return in_maps


def _assemble(blocks) -> np.ndarray:
    """Place each core's [1024, 4224] fp16 bands, then mirror the rest."""
    full = np.empty((N, N), dtype=np.float32)
    for c, blk in enumerate(blocks):
        for m in range(NT_M):
            start = m * NCORES * PT + c * PT
            rows = slice(start, start + PT)
            part = blk[m * PT : (m + 1) * PT, :]
            end = start + W
            if end <= N:
                full[rows, start:end] = part
            else:
                k = N - start
                full[rows, start:] = part[:, :k]
                full[rows, : end - N] = part[:, k:]
    # Mirror the uncomputed [128, 128] blocks from their transposes.
    nw = W // PT  # 33 directly-written blocks per block row
    for r in range(NBLK):
        for s in range(NBLK):
            if (s - r) % NBLK >= nw:
                full[r * PT : (r + 1) * PT, s * PT : (s + 1) * PT] = full[
                    s * PT : (s + 1) * PT, r * PT : (r + 1) * PT
                ].T
    np.fill_diagonal(full, 0.0)
    return full


def _run(inputs, trace=False, trace_cores=None):
    from concourse.bass_utils import run_bass_kernel_spmd

    x = np.ascontiguousarray(np.asarray(inputs["x"], dtype=np.float32))
    assert x.shape == (N, D), x.shape
    in_maps = _prep_inputs(x)
    res = run_bass_kernel_spmd(
        _get_nc(),
        in_maps,
        core_ids=list(range(NCORES)),
        trace=trace,
        trace_cores=trace_cores,
    )
    full = _assemble([r["out"] for r in res.results])
    return full, res


def kernel(**inputs) -> np.ndarray:
    full, _ = _run(inputs)
    return full
